# revision 154
# baseline (speedup 1.0000x reference)
"""Trainium2 Bass kernel for nn_Attention4D (EfficientViT-style attention).

Strategy (8 NeuronCores, data-parallel over batch B=8, one element per core):
  - BN folded into conv weights on host; talking-head-1 folded into
    per-head-scaled queries (Q2) so q@k contracts 256 channels.
  - Scale management so fp8e4m3 tensors stay inside the normal range
    (the tiny softmax/Wth2 products otherwise flush to subnormal zero):
      * q2/abt carry x16, removed by the exp activation's scale=1/16
      * w2bd carries x1024 (S) so w2s = w2bd*(1/z) and a2T are healthy fp8;
        the 1/S rides the osum psum->sbuf copy (scale=1/S, bias=bias2)
      * vpad (depthwise input) carries x8, taps carry S/8=128
  - fp8 + MatmulPerfMode.DoubleRow (0.5 cyc/row) everywhere on the PE:
      * q/k/vT projections: x8 (fp8, 4 padded k-tiles; biases ride the
        pad tile: x8 tile3 ch0 = 1, weight tile3 row0 = bias)
      * logits: one DoubleRow (K=256) per 392-col chunk + [I|0] rel-pos add
      * talking-head-2: a2T = e_chunk^T @ w2s per (group, key-chunk)
      * depthwise 3x3: 5 tap-pairs (9 taps + zero) per group
      * attn@V: vT key-tile pairs (8 tiles, tile 7 zeroed) -> 4 DoubleRows
    (the depthwise V stays bf16: its fp8-input matmul noise lands on the
    output directly and breaks the tolerance)
    a2T layout is (g, pair, member, query) so every matmul AP stays at
    <=3 free dims; a2T m=7 / vT tiles 6/7 are zeroed via DMA from a
    zeros input so the pair reads stay finite at no engine cost.
  - psum->sbuf copy work is balanced across ACT/DVE/GpSimd by a
    least-loaded heuristic with per-engine cost models; exp (ACT-only)
    dominates ACT so the balancer steers copies to DVE/GpSimd.
  - Phase C is software-pipelined (depth 2); v/vt projections and phase-D
    head blocks are interleaved into the group loop as PE filler.
  - Phase D is split into three query chunks (448/224/112) so the first
    two start inside the group loop as soon as their a2T columns land.
  - Dummy matmuls during the initial DMA wait pre-ramp the PE clock.
"""

import os
import sys

sys.path.insert(0, "/opt/trn_rl_repo")

_DEBUG = os.environ.get("KERNEL_DEBUG") == "1"

import numpy as np
import ml_dtypes

import concourse.bass as bass
import concourse.tile as tile
from concourse import bacc, mybir
from concourse.bass_utils import run_bass_kernel_spmd
from bass_rust import AP

F32 = mybir.dt.float32
BF16 = mybir.dt.bfloat16
FP8 = mybir.dt.float8e4
AF = mybir.ActivationFunctionType
ALU = mybir.AluOpType
DR = mybir.MatmulPerfMode.DoubleRow
BF = ml_dtypes.bfloat16
F8 = ml_dtypes.float8_e4m3

HEADS, KD, AR, RES, DIM = 8, 32, 4, 28, 384
D = AR * KD            # 128
DH = HEADS * D         # 1024
NH_KD = HEADS * KD     # 256
N = RES * RES          # 784
NG = N // 16           # 49 groups of 16 queries
B = 8

SQ = 16.0              # logits scale (q2/abt), removed by exp scale
S_A2 = 1024.0          # a2T scale (in w2bd), removed in osum copy
VS = 8.0               # vpad scale; taps carry S_A2/VS
SINV = 1.0 / S_A2

_CACHE = {}
LAST_RESULTS = None  # test.py reads exec_time from here


def _sv(base, off_elems, dims):
    """Custom strided view: keep base AP's partition dim, free dims given as
    [(stride, size), ...]."""
    ap = [list(base.ap[0])] + [[s, n] for (s, n) in dims]
    return AP(base.tensor, base.offset + off_elems, ap)


def _build_program():
    nc = bacc.Bacc("TRN2", target_bir_lowering=False, debug=False,
                   enable_asserts=True)

    def din(name, shape, dt=F32):
        return nc.dram_tensor(name, shape, dt, kind="ExternalInput")

    x_c = din("x_c", [128, 3 * N], BF16)
    x8d = din("x8d", [128, 4 * N], FP8)
    wq8d = din("wq8d", [128, 4 * NH_KD], FP8)
    wk8d = din("wk8d", [128, 4 * NH_KD], FP8)
    wv3 = din("wv3", [128, 3 * DH], BF16)
    wv8d = din("wv8d", [128, 4 * DH], FP8)
    wp8 = din("wp8", [128, 8 * DIM], BF16)
    vecs = din("vecs", [128, 448])   # [bias vecs | w2bd | bq*sq | sq-pat]
    idz = din("idz", [128, 256], FP8)
    dwp = din("dwp", [128, 8 * 5 * 256], FP8)
    abt = din("abt", [128, NG * N], FP8)
    z8d = din("z8d", [128, 6272], FP8)

    out = nc.dram_tensor("out", [DIM, N], F32, kind="ExternalOutput")

    CH = (slice(0, 392), slice(392, 784))
    PSC = (slice(0, 392), slice(512, 904))

    def psum2view(ps):
        return ps[:].rearrange("p (a c) -> p a c", c=512)[:, :, 0:392]

    # engine-balance bookkeeping for flexible ACT/DVE/GpSimd ops
    ebusy = {"act": 0.0, "dve": 0.0, "pool": 0.0}

    def _costs(free, accum=False):
        # small ACT penalty: the exps keep ACT the most contended engine
        return {"act": (free * 0.833 + 190) * 1.03 + (187 if accum else 0),
                "dve": free * 1.042 + 130,
                "pool": free * 1.389 + 130}

    def flex(free, act_fn, dve_fn, pool_fn=None, accum=False):
        c = _costs(free, accum)
        cands = [("act", act_fn), ("dve", dve_fn)]
        if pool_fn is not None:
            cands.append(("pool", pool_fn))
        eng, fn = min(cands, key=lambda kv: ebusy[kv[0]] + c[kv[0]])
        ebusy[eng] += c[eng]
        fn()

    def act_only(free, fn, accum=False):
        ebusy["act"] += _costs(free, accum)["act"]
        fn()

    def dve_only(free, fn):
        ebusy["dve"] += _costs(free)["dve"]
        fn()

    def pool_only(free, fn):
        ebusy["pool"] += _costs(free)["pool"]
        fn()

    with tile.TileContext(nc) as tc:
        with (
            tc.tile_pool(name="consts", bufs=1) as consts,
            tc.tile_pool(name="persist", bufs=1) as persist,
        ):
            vec_t = consts.tile([128, 448], F32, tag="vec_t", name="vec_t")
            bq_t = [vec_t[:, k:k + 1] for k in range(2)]
            bk_t = [vec_t[:, 2 + k:3 + k] for k in range(2)]
            bv_t = [vec_t[:, 4 + k:5 + k] for k in range(8)]
            bdw2_t = [vec_t[:, 12 + k:13 + k] for k in range(8)]
            bp_t = [vec_t[:, 20 + k:21 + k] for k in range(3)]
            bth1_t = vec_t[:, 23:24]
            bth2g_t = [vec_t[:, 24 + k:25 + k] for k in range(8)]
            sq_t = [[vec_t[:, 32 + kt * 8 + g:33 + kt * 8 + g]
                     for g in range(8)] for kt in range(2)]
            bqsq_t = [[vec_t[:, 176 + kt * 8 + g:177 + kt * 8 + g]
                       for g in range(8)] for kt in range(2)]
            w2bd_t = vec_t[:, 48:176]
            idz_t = consts.tile([128, 256], FP8, tag="idz", name="idz")
            dwp_t = consts.tile([128, 8 * 5 * 256], FP8, tag="dwp",
                                name="dwp")
            wp_w = consts.tile([128, 8 * DIM], BF16, tag="wp_w", name="wp_w")
            wp_t = [wp_w[:, k * DIM:(k + 1) * DIM] for k in range(8)]
            x8_t = consts.tile([128, 4 * N], FP8, tag="x8", name="x8")
            wv8_t = consts.tile([128, 4 * DH], FP8, tag="wv8", name="wv8")
            wq8_t = consts.tile([128, 4 * NH_KD], FP8, tag="wq8",
                                name="wq8")
            wk8_t = consts.tile([128, 4 * NH_KD], FP8, tag="wk8",
                                name="wk8")

            # persistent activations
            q2 = persist.tile([128, 2 * NG * 128], FP8, tag="q2", name="q2")
            q2tv = q2[:].rearrange("p (t c) -> p t c", c=NG * 128)
            k2 = persist.tile([128, 2 * N], FP8, tag="k2", name="k2")
            k2v = k2[:].rearrange("p (t c) -> p t c", c=N)
            vpad = [persist.tile([128, 900], FP8, tag=f"vpad{p}",
                                 name=f"vpad{p}") for p in range(8)]
            vt8 = persist.tile([128, 8 * DH], FP8, tag="vt8", name="vt8")
            # a2T layout: cols = g*6272 + pair*1568 + member*784 + n
            a2t = persist.tile([128, 8 * 6272], FP8, tag="a2t", name="a2t")
            # per-chunk osum tiles so phase-D projections of chunk A never
            # falsely serialize behind chunk B/C writes
            osum = [[persist.tile([128, w], BF16, tag=f"osum{ci}_{p}",
                                  name=f"osum{ci}_{p}") for p in range(8)]
                    for ci, w in enumerate((448, 224, 112))]
            vsum = persist.tile([128, 8], F32, tag="vsum", name="vsum")
            bias2 = persist.tile([128, 8], F32, tag="bias2", name="bias2")
            qb = [persist.tile([128, N], BF16, tag=f"qb{k}", name=f"qb{k}")
                  for k in range(2)]

            with (
                tc.tile_pool(name="ax", bufs=1) as axpool,
                tc.tile_pool(name="pbig", bufs=2, space="PSUM") as pbig,
                tc.tile_pool(name="pa2", bufs=2, space="PSUM") as pa2,
                tc.tile_pool(name="cw", bufs=1) as cw,
                tc.tile_pool(name="cz", bufs=3) as cz,
            ):
                # ---------------- loads ---------------------------------
                # DMA order is latency-tuned: wq + the x ci=0 halves gate
                # the first projection matmul; everything else follows.
                nc.sync.dma_start(vec_t[:], vecs.ap()[:])
                # PE p-state warm-up: dummy matmuls on a zeroed tile while
                # the first DMAs land, so projections start near full clock
                wuz = axpool.tile([128, 256], BF16, tag="wuz", name="wuz")
                nc.vector.memset(wuz[:], 0.0)
                wups = pbig.tile([128, 1024], F32, tag="big", name="big")
                for _w in range(5):
                    nc.tensor.matmul(wups[:, 0:512], lhsT=wuz[:, 0:128],
                                     rhs=_sv(wuz[:], 0, [(0, 512)]),
                                     start=True, stop=True)
                x_w = axpool.tile([128, 3 * N], BF16, tag="xw", name="xw")

                def load_a(name, src, shape, dt):
                    t = axpool.tile(shape, dt, tag=name, name=name)
                    nc.sync.dma_start(t[:], src)
                    return t

                # projections only need the small fp8 loads: wq8, x8, wk8
                nc.sync.dma_start(wq8_t[:], wq8d.ap()[:])
                nc.scalar.dma_start(x8_t[:], x8d.ap()[:])
                nc.sync.dma_start(wk8_t[:], wk8d.ap()[:])
                nc.scalar.dma_start(idz_t[:], idz.ap()[:])
                x_t = [x_w[:, k * N:(k + 1) * N] for k in range(3)]

                # pre-zero the rotating-buffer slack regions on the idle
                # GpSimd so th2T reads of e[:,784:1024] / w2s[:,128:256] /
                # ab slack are defined
                for _i in range(4):
                    _e = cw.tile([128, 1024], FP8, tag="e", name="e", bufs=4)
                    nc.gpsimd.memset(_e[:, 784:1024], 0.0)
                    _w = cw.tile([128, 256], FP8, tag="w2s", name="w2s",
                                 bufs=4)
                    nc.gpsimd.memset(_w[:, 128:256], 0.0)
                abtiles = {}

                def fetch_ab(kb, half=None):
                    nab = min(8, NG - kb * 8)
                    if half is None:
                        t = cw.tile([128, 8 * N + 392], FP8, tag="ab",
                                    name="ab", bufs=3)
                        nc.sync.dma_start(
                            t[:, 0:nab * N],
                            abt.ap()[:, kb * 8 * N:(kb * 8 + nab) * N])
                        abtiles[kb] = t
                    elif half == 0:
                        t = cw.tile([128, 8 * N + 392], FP8, tag="ab",
                                    name="ab", bufs=3)
                        h = (nab + 1) // 2
                        nc.sync.dma_start(
                            t[:, 0:h * N],
                            abt.ap()[:, kb * 8 * N:kb * 8 * N + h * N])
                        abtiles[kb] = t
                    else:
                        t = abtiles[kb]
                        h = (nab + 1) // 2
                        nc.sync.dma_start(
                            t[:, h * N:nab * N],
                            abt.ap()[:, kb * 8 * N + h * N:
                                     (kb * 8 + nab) * N])

                for _i in range(3):
                    _a = cw.tile([128, 8 * N + 392], FP8, tag="ab",
                                 name="ab", bufs=3)
                    nc.gpsimd.memset(_a[:, 8 * N:], 0.0)
                fetch_ab(0, half=0)
                nc.sync.dma_start(x_w[:], x_c.ap()[:])
                wv_w = load_a("wv_w", wv3.ap()[:], [128, 3 * DH], BF16)
                wv_t = [wv_w[:, k * DH:(k + 1) * DH] for k in range(3)]
                fetch_ab(0, half=1)
                fetch_ab(1)
                nc.sync.dma_start(wv8_t[:], wv8d.ap()[:])
                nc.sync.dma_start(dwp_t[:], dwp.ap()[:])
                # zero the fp8 pair-padding regions via DMA (no engine
                # cost): vT tiles 6/7 and the a2T (pair3, member1) columns
                nc.sync.dma_start(vt8[:, 6 * DH:8 * DH],
                                  z8d.ap()[:, 0:2 * DH])
                nc.sync.dma_start(
                    _sv(a2t[:], 5488, [(6272, 8), (1, 784)]),
                    z8d.ap()[:].rearrange("p (a c) -> p a c", c=784))

                # q2 fp8 prep, engine-flexible, batched by group ranges;
                # batches 0/1 are emitted inline right after each qb copy
                # ((0,4) tiny so logits_0 isn't gated on a long emit train)
                q2_ops = []
                for (a0, a1) in ((12, 26), (26, 38), (38, 49)):
                    for kt in range(2):
                        for g in range(8):
                            q2_ops.append((kt, g, a0, a1))

                def emit_q2(kt, g, a0, a1):
                    dst = _sv(q2[:], kt * NG * 128 + a0 * 128 + g * 16,
                              [(128, a1 - a0), (1, 16)])
                    src = _sv(qb[kt][:], a0 * 16, [(16, a1 - a0), (1, 16)])
                    sct = sq_t[kt][g]
                    flex((a1 - a0) * 16,
                         lambda: nc.scalar.activation(
                             dst, src, AF.Copy, bias=0.0, scale=sct),
                         lambda: nc.vector.tensor_scalar_mul(dst, src, sct),
                         lambda: nc.gpsimd.tensor_scalar_mul(dst, src, sct))

                def emit_q2_tt(kt, a0, a1, pool=False):
                    # whole group-range in ONE op: q2 = qb (broadcast over
                    # the 8 out-heads via a stride-0 dim) * sq-pattern
                    dst = _sv(q2[:], kt * NG * 128 + a0 * 128,
                              [(128, a1 - a0), (16, 8), (1, 16)])
                    src = _sv(qb[kt][:], a0 * 16,
                              [(16, a1 - a0), (0, 8), (1, 16)])
                    sqp = _sv(vec_t[:], 192 + kt * 128,
                              [(0, a1 - a0), (16, 8), (1, 16)])
                    if pool:
                        pool_only((a1 - a0) * 128 * 1.43,
                                  lambda: nc.gpsimd.tensor_mul(
                                      dst, src, sqp))
                    else:
                        dve_only((a1 - a0) * 128,
                                 lambda: nc.vector.tensor_mul(
                                     dst, src, sqp))

                # ---------------- q/k projections ------------------------
                # fp8 DoubleRow (biases ride the padding k-tile: x8 tile3
                # ch0 = 1, weight tile3 row0 = bias); q uses pbig, k the
                # still-idle pa2 pool so all four stages pipeline
                kps = {}
                for (w8t, fp8out) in ((wq8_t, False), (wk8_t, True)):
                    for ot in range(2):
                        ppool, ptag = ((pa2, "a2ps") if fp8out
                                       else (pbig, "big"))
                        ps = ppool.tile([128, 1024], F32, tag=ptag,
                                        name=ptag)
                        for ci in range(2):
                            pchunk = ps[:, PSC[ci]]
                            for pr in range(2):
                                nc.tensor.matmul(
                                    pchunk,
                                    lhsT=_sv(w8t[:],
                                             pr * 2 * NH_KD + ot * 128,
                                             [(NH_KD, 2), (1, 128)]),
                                    rhs=_sv(x8_t[:], pr * 2 * N + ci * 392,
                                            [(N, 2), (1, 392)]),
                                    start=(pr == 0), stop=(pr == 1),
                                    perf_mode=DR)
                        if fp8out:
                            kps[ot] = ps
                        else:
                            dst = qb[ot][:].rearrange(
                                "p (a c) -> p a c", c=392)
                            if ot == 0:
                                act_only(784, lambda d=dst, p=ps:
                                         nc.scalar.copy(d, psum2view(p)))
                            else:
                                dve_only(784, lambda d=dst, p=ps:
                                         nc.vector.tensor_scalar_add(
                                             d, psum2view(p), 0.0))
                            emit_q2_tt(ot, 0, 4)
                # k casts: chunk-0 of both k-tiles first (gates logits_0),
                # split across ACT/DVE
                for ci in range(2):
                    for ot in range(2):
                        dst = _sv(k2[:], ot * N + ci * 392, [(1, 392)])
                        ps = kps[ot]
                        if ot == 0:
                            act_only(392, lambda d=dst, p=ps, c=ci:
                                     nc.scalar.copy(d, p[:, PSC[c]]))
                        else:
                            dve_only(392, lambda d=dst, p=ps, c=ci:
                                     nc.vector.tensor_scalar_add(
                                         d, p[:, PSC[c]], 0.0))
                emit_q2_tt(0, 4, 12)
                emit_q2_tt(1, 4, 12)

                # ---------------- filler task queues ---------------------
                def v_task(p):
                    def go():
                        vvz = vpad[p][:].rearrange("p (r c) -> p r c", c=30)
                        nc.gpsimd.memset(vvz[:, 0, :], 0.0)
                        nc.gpsimd.memset(vvz[:, 29, :], 0.0)
                        nc.gpsimd.memset(vvz[:, 1:29, 0], 0.0)
                        nc.gpsimd.memset(vvz[:, 1:29, 29], 0.0)
                        ps = pa2.tile([128, 1024], F32, tag="a2ps",
                                      name="a2ps")
                        for ci in range(2):
                            pchunk = ps[:, PSC[ci]]
                            for kt in range(3):
                                nc.tensor.matmul(
                                    pchunk,
                                    lhsT=wv_t[kt][:, p * 128:(p + 1) * 128],
                                    rhs=x_t[kt][:, CH[ci]],
                                    start=(kt == 0), stop=(kt == 2))
                        # DVE-pinned: out = VS*pin + VS*bv; tensor_scalar
                        # accumulates after op0 only, so vsum = VS*sum(v_raw)
                        # and the missing 784*bth2*bv ride bdw2 on the host
                        rows = vvz[:, 1:29, 1:29].rearrange(
                            "p (a r) c -> p a r c", a=2)
                        pin = psum2view(ps).rearrange(
                            "p a (r c) -> p a r c", c=28)
                        vs = vsum[:, p:p + 1]
                        dve_only(784, lambda: nc.vector.tensor_scalar(
                            rows, pin, VS, bv_t[p], ALU.mult,
                            ALU.add, accum_out=vs))
                        if p == 7:
                            for g in range(8):
                                nc.vector.scalar_tensor_tensor(
                                    bias2[:, g:g + 1], vsum[:, g:g + 1],
                                    bth2g_t[g], bdw2_t[g],
                                    ALU.mult, ALU.add)
                    return go

                def vt_task(mt):
                    def go():
                        M = 128 if mt < 6 else 16
                        ps = pa2.tile([128, 1024], F32, tag="a2ps",
                                      name="a2ps")
                        for ci in range(2):
                            pchunk = ps[0:M, ci * 512:(ci + 1) * 512]
                            for pr in range(2):
                                nc.tensor.matmul(
                                    pchunk,
                                    lhsT=_sv(x8_t[:], pr * 2 * N + mt * 128,
                                             [(N, 2), (1, M)]),
                                    rhs=_sv(wv8_t[:], pr * 2 * DH + ci * 512,
                                            [(DH, 2), (1, 512)]),
                                    start=(pr == 0), stop=(pr == 1),
                                    perf_mode=DR)
                        dst = vt8[0:M, mt * DH:(mt + 1) * DH]
                        flex(1024,
                             lambda p=ps, m=M:
                             nc.scalar.copy(dst, p[0:m, :]),
                             lambda p=ps, m=M:
                             nc.vector.tensor_scalar_add(
                                 dst, p[0:m, :], 0.0))
                    return go

                DCH = ((0, 16, 0, 28, 448), (16, 8, 28, 42, 224),
                       (24, 4, 42, NG, 112))

                def dw_attnv(g, ci, pool, tag, tilew=1024):
                    r0, nr, a0, a1, w = DCH[ci]
                    po = pool.tile([128, tilew], F32, tag=tag, name=tag)
                    pov = po[:, 0:w]
                    vflat = vpad[g][:]
                    for p in range(4):
                        t0 = 2 * p
                        dy0, dx0 = t0 // 3, t0 % 3
                        dy1, dx1 = (t0 + 1) // 3, (t0 + 1) % 3
                        delta = (dy1 - dy0) * 30 + (dx1 - dx0)
                        rhs = _sv(vflat, (r0 + dy0) * 30 + dx0,
                                  [(delta, 2), (30, nr), (1, 28)])
                        lw = dwp_t[:, g * 1280 + p * 256:
                                   g * 1280 + p * 256 + 256].rearrange(
                                       "p (t c) -> p t c", c=128)
                        nc.tensor.matmul(pov, lhsT=lw, rhs=rhs,
                                         start=(p == 0), stop=False,
                                         perf_mode=DR)
                    # pair 4: (tap 8, zero) with a stride-0 window pair
                    rhs8 = _sv(vflat, (r0 + 2) * 30 + 2,
                               [(0, 2), (30, nr), (1, 28)])
                    lw4 = dwp_t[:, g * 1280 + 1024:
                                g * 1280 + 1280].rearrange(
                                    "p (t c) -> p t c", c=128)
                    nc.tensor.matmul(pov, lhsT=lw4, rhs=rhs8,
                                     start=False, stop=False, perf_mode=DR)
                    # attn @ V: 4 key-tile-pair DoubleRows
                    for t in range(4):
                        rhs = _sv(a2t[:], g * 6272 + t * 1568 + r0 * 28,
                                  [(784, 2), (1, w)])
                        nc.tensor.matmul(
                            pov,
                            lhsT=_sv(vt8[:], t * 2 * DH + g * 128,
                                     [(DH, 2), (1, 128)]),
                            rhs=rhs, start=False, stop=(t == 3),
                            perf_mode=DR)
                    dst = osum[ci][g][:]
                    if ci == 0:
                        # the in-loop dw iterations leave ~600ns ACT
                        # bubbles while DVE carries th2T + this copy;
                        # pin it to ACT to fill them
                        act_only(w, lambda: nc.scalar.activation(
                            dst, pov, AF.Identity,
                            bias=bias2[:, g:g + 1], scale=SINV))
                    else:
                        flex(w,
                             lambda: nc.scalar.activation(
                                 dst, pov, AF.Identity,
                                 bias=bias2[:, g:g + 1], scale=SINV),
                             lambda: nc.vector.tensor_scalar(
                                 dst, pov, SINV, bias2[:, g:g + 1],
                                 ALU.mult, ALU.add))

                tasks_early = [v_task(p) for p in range(8)] + \
                              [vt_task(mt) for mt in range(7)]
                tasks_late = [(lambda gg=g: dw_attnv(gg, 0, pa2, "a2ps"))
                              for g in range(8)]
                tasks_lateb = [(lambda gg=g: dw_attnv(gg, 1, pa2, "a2ps"))
                               for g in range(5)]

                # ---------------- phase C group loop ---------------------
                pending = []

                def th2t_and_copy(gi, e_t, w2s_t):
                    a2ps = pa2.tile([128, 1024], F32, tag="a2ps",
                                    name="a2ps")
                    w2sv = w2s_t[:].rearrange("p (t c) -> p t c", c=128)
                    for m in range(7):
                        # k-tile pair [e chunk; zeros at col 896]
                        lhsT = _sv(e_t[:], m * 128,
                                   [(896 - m * 128, 2), (1, 128)])
                        nc.tensor.matmul(
                            a2ps[:, m * 128:(m + 1) * 128],
                            lhsT=lhsT, rhs=w2sv,
                            start=True, stop=True, perf_mode=DR)
                    src = a2ps[:, 0:896].rearrange(
                        "p (m g q) -> p m g q", g=8, q=16)
                    dst = _sv(a2t[:], gi * 16,
                              [(784, 7), (6272, 8), (1, 16)])
                    flex(896,
                         lambda: nc.scalar.copy(dst, src),
                         lambda: nc.vector.tensor_scalar_add(dst, src, 0.0))

                for gi in range(NG):
                    if gi % 8 == 4 and gi // 8 + 2 <= (NG - 1) // 8:
                        fetch_ab(gi // 8 + 2)
                    ab4 = abtiles[gi // 8]
                    j = gi % 8

                    lg = pbig.tile([128, 1024], F32, tag="big", name="big")
                    for ci in range(2):
                        pchunk = lg[:, PSC[ci]]
                        nc.tensor.matmul(
                            pchunk, lhsT=q2tv[:, :, gi * 128:(gi + 1) * 128],
                            rhs=k2v[:, :, CH[ci]],
                            start=True, stop=False, perf_mode=DR)
                        abrhs = _sv(ab4[:], j * N + ci * 392,
                                    [(392, 2), (1, 392)])
                        nc.tensor.matmul(
                            pchunk,
                            lhsT=idz_t[:].rearrange("p (t c) -> p t c",
                                                    c=128),
                            rhs=abrhs,
                            start=False, stop=True, perf_mode=DR)

                    e_t = cw.tile([128, 1024], FP8, tag="e", name="e",
                                  bufs=4)
                    z = cz.tile([128, 1], F32, tag="z", name="z")
                    ev = e_t[:, 0:784].rearrange("p (a c) -> p a c", c=392)
                    act_only(784, lambda: nc.scalar.activation(
                        ev, psum2view(lg), AF.Exp, bias=bth1_t,
                        scale=1.0 / SQ, accum_out=z[:]), accum=True)

                    w2s_t = cw.tile([128, 256], FP8, tag="w2s", name="w2s",
                                    bufs=4)
                    r = cz.tile([128, 1], F32, tag="r", name="r")
                    nc.vector.reciprocal(r[:], z[:])
                    # early groups: keep w2s on DVE right behind reciprocal
                    # (the list scheduler hoists ready q2 work ahead of it
                    # in Pool's in-order queue otherwise)
                    c_d, c_p = _costs(128)["dve"], _costs(128)["pool"]
                    if gi < 14 or ebusy["dve"] + c_d < ebusy["pool"] + c_p:
                        dve_only(128, lambda: nc.vector.tensor_scalar_mul(
                            w2s_t[:, 0:128], w2bd_t, r[:]))
                    else:
                        pool_only(128, lambda: nc.gpsimd.tensor_scalar_mul(
                            w2s_t[:, 0:128], w2bd_t, r[:]))

                    pending.append((gi, e_t, w2s_t))
                    if len(pending) > 2:
                        th2t_and_copy(*pending.pop(0))

                    # fillers (start after the pipeline is primed and the
                    # big weight DMAs have landed)
                    if 0 <= gi < 28:
                        if tasks_early and (gi * 15) // 27 > \
                                14 - len(tasks_early):
                            tasks_early.pop(0)()
                    elif gi >= 31 and tasks_late and (gi - 31) % 2 == 0:
                        tasks_late.pop(0)()
                    elif gi >= 44 and tasks_lateb:
                        tasks_lateb.pop(0)()
                    for _ in range(2):
                        if q2_ops:
                            emit_q2(*q2_ops.pop(0))

                while pending or tasks_early or tasks_late or \
                        tasks_lateb:
                    if pending:
                        th2t_and_copy(*pending.pop(0))
                    if tasks_early:
                        tasks_early.pop(0)()
                    if tasks_late:
                        tasks_late.pop(0)()
                    if tasks_lateb:
                        tasks_lateb.pop(0)()

                # ============= phase D tail ==========================
                # stays inside the same psum pools: pp accumulators live in
                # pbig (free right after the last exp) and the remaining dw
                # head-blocks keep using the pa2 rotation, so nothing waits
                # on a whole-pool handover
                nc.sync.dma_start(wp_w[:], wp8.ap()[:])
                # one ot tile holding all 3 row-blocks side by side so each
                # chunk ships as a single strided DMA
                otw = axpool.tile([128, 3 * N], F32, tag="otw", name="otw")

                def pp_chunk(ci, interleave=None):
                    r0, nr, a0, a1, w = DCH[ci]
                    c0 = r0 * 28
                    ps1 = pbig.tile([128, 1024], F32, tag="big", name="big")
                    ps2 = pbig.tile([128, 1024], F32, tag="big", name="big")
                    pp = [ps1[:, 0:448], ps1[:, 512:960], ps2[:, 0:448]]
                    for g in range(8):
                        if interleave is not None and g < len(interleave):
                            interleave[g]()
                        for mt in range(3):
                            nc.tensor.matmul(
                                pp[mt][:, 0:w],
                                lhsT=wp_t[g][:, mt * 128:(mt + 1) * 128],
                                rhs=osum[ci][g][:],
                                start=(g == 0), stop=(g == 7))
                    for mt in range(3):
                        dst = _sv(otw[:], mt * N + c0, [(1, w)])
                        flex(w,
                             lambda m=mt, d=dst: nc.scalar.activation(
                                 d, pp[m][:, 0:w], AF.Identity,
                                 bias=bp_t[m]),
                             lambda m=mt, d=dst: nc.vector.tensor_scalar_add(
                                 d, pp[m][:, 0:w], bp_t[m]))
                    oap = out.ap()
                    nc.sync.dma_start(
                        AP(oap.tensor, oap.offset + c0,
                           [[N, 128], [128 * N, 3], [1, w]]),
                        _sv(otw[:], c0, [(N, 3), (1, w)]))

                # chunk A projection (everything ready), with the remaining
                # B-chunk head blocks interleaved between its g-steps
                remb = [(lambda gg=g: dw_attnv(gg, 1, pa2, "a2ps"))
                        for g in range(5, 8)]
                pp_chunk(0, interleave=remb)
                # chunk B projection with C2 head blocks interleaved
                remc = [(lambda gg=g: dw_attnv(gg, 2, pa2, "a2ps"))
                        for g in range(8)]
                pp_chunk(1, interleave=remc)
                # chunk C2 projection
                pp_chunk(2)

                if _DEBUG:
                    for nm, t, cols, dt_ in (
                            ("dbg_q2", q2, 2 * NG * 128, FP8),
                            ("dbg_k2", k2, 2 * N, FP8),
                            ("dbg_vt8", vt8, 8 * DH, FP8),
                            ("dbg_a2t", a2t, 8 * 6272, FP8),
                            ("dbg_vp0", vpad[0], 900, FP8),
                            ("dbg_os0", osum[0][0], 448, BF16),
                            ("dbg_vsum", vsum, 8, F32),
                            ("dbg_bias2", bias2, 8, F32)):
                        dto = nc.dram_tensor(nm, [128, cols], dt_,
                                             kind="ExternalOutput")
                        nc.sync.dma_start(dto.ap()[:], t[:])

    nc.compile()
    return nc


def _prep_common(inputs):
    f32 = np.float32
    scale = np.float32(KD ** -0.5)
    q_s, q_b = inputs["q_s"], inputs["q_b"]
    k_s, k_b = inputs["k_s"], inputs["k_b"]
    v_s, v_b = inputs["v_s"], inputs["v_b"]
    p_s, p_b = inputs["p_s"], inputs["p_b"]

    Wq = np.asarray(inputs["Wq"], f32) * np.asarray(q_s, f32)[:, None] * scale
    bqv = (np.asarray(q_s, f32) * np.asarray(inputs["bq"], f32)
           + np.asarray(q_b, f32)) * scale
    Wk = np.asarray(inputs["Wk"], f32) * np.asarray(k_s, f32)[:, None]
    bkv = np.asarray(k_s, f32) * np.asarray(inputs["bk"], f32) \
        + np.asarray(k_b, f32)
    Wv = np.asarray(inputs["Wv"], f32) * np.asarray(v_s, f32)[:, None]
    bvv = np.asarray(v_s, f32) * np.asarray(inputs["bv"], f32) \
        + np.asarray(v_b, f32)
    Wp = np.asarray(inputs["Wp"], f32) * np.asarray(p_s, f32)[:, None]
    bpv = np.asarray(p_s, f32) * np.asarray(inputs["bp"], f32) \
        + np.asarray(p_b, f32)

    Wth1 = np.asarray(inputs["Wth1"], f32)
    bth1 = np.asarray(inputs["bth1"], f32)
    Wth2 = np.asarray(inputs["Wth2"], f32)
    bth2 = np.asarray(inputs["bth2"], f32)

    # th1-folded rel-pos bias table (x SQ), rows (group, g, i)
    ab1 = Wth1 @ np.asarray(inputs["attention_biases"], f32)
    idx = np.asarray(inputs["bias_idxs"])
    ab_full = ab1[:, idx] * SQ                             # [8,784,784]
    abt = np.ascontiguousarray(
        ab_full.reshape(8, NG, 16, N).transpose(1, 0, 2, 3)
    ).reshape(NG, 128, N).transpose(1, 0, 2).reshape(128, NG * N)
    abt = np.ascontiguousarray(abt).astype(F8)

    # depthwise weights folded with BN -> fp8 diagonal pair blocks
    # (5 DoubleRow pairs per group: taps (0,1)(2,3)(4,5)(6,7)(8,zero)),
    # scaled by S_A2/VS to match the vpad x VS input scaling
    wvl = np.asarray(inputs["Wvl"], f32)[:, 0, :, :].reshape(DH, 9)
    vl_s = np.asarray(inputs["vl_s"], f32)
    wtap = wvl * vl_s[:, None] * (S_A2 / VS)               # [1024, 9]
    bdw = (np.asarray(inputs["bvl"], f32) * vl_s
           + np.asarray(inputs["vl_b"], f32))
    dwp = np.zeros((128, 8 * 5 * 256), f32)
    for g in range(8):
        for t in range(9):
            blk = np.zeros((128, 128), f32)
            np.fill_diagonal(blk, wtap[g * 128:(g + 1) * 128, t])
            dwp[:, g * 1280 + t * 128:g * 1280 + (t + 1) * 128] = blk

    def ktile_pack(wT, nk):
        C = wT.shape[1]
        return np.ascontiguousarray(
            wT.reshape(nk, 128, C).transpose(1, 0, 2).reshape(128, nk * C))

    sqv = np.repeat(Wth1.T, KD, axis=0).astype(f32) * SQ   # [256, 8]
    vecs = np.zeros((128, 448), f32)
    vecs[:, 0:2] = bqv.reshape(2, 128).T
    vecs[:, 2:4] = bkv.reshape(2, 128).T
    vecs[:, 4:12] = (bvv * VS).reshape(8, 128).T
    # vT's bias rides the fp8 V-matmul padding tile (no s2p correction);
    # vsum misses the bias term, so 784*bth2*bv folds in here
    bdw2 = bdw + float(N) * np.repeat(bth2, D) * bvv
    vecs[:, 12:20] = bdw2.reshape(8, 128).T
    vecs[:, 20:23] = bpv.reshape(3, 128).T
    vecs[:, 23] = np.repeat(bth1, 16)
    for g in range(8):
        vecs[:, 24 + g] = bth2[g] / VS
    vecs[:, 32:40] = sqv[0:128]
    vecs[:, 40:48] = sqv[128:256]
    vecs[:, 48:176] = np.kron(Wth2.T, np.eye(16, dtype=f32)) * S_A2
    for kt in range(2):
        for g in range(8):
            vecs[:, 176 + kt * 8 + g] = \
                bqv[kt * 128:(kt + 1) * 128] * sqv[kt * 128:(kt + 1) * 128, g]
        # sq pattern for the one-op q2 emits: col g*16+q = sq[c, g]
        vecs[:, 192 + kt * 128:192 + (kt + 1) * 128] = \
            np.repeat(sqv[kt * 128:(kt + 1) * 128], 16, axis=1)

    # q/k/v weights: fp8 k-tile-pair pack, K padded 384 -> 512; the first
    # padding row carries the bias (paired with x8 tile3 ch0 = 1)
    def pad_pack8(W, b, o):
        WT = np.zeros((512, o), f32)
        WT[0:DIM] = W.T
        WT[DIM] = b
        return ktile_pack(np.ascontiguousarray(WT), 4).astype(F8)

    common = {
        "wq8d": pad_pack8(Wq, bqv, NH_KD),
        "wk8d": pad_pack8(Wk, bkv, NH_KD),
        "wv3": ktile_pack(np.ascontiguousarray(Wv.T), 3).astype(BF),
        "wv8d": pad_pack8(Wv, bvv, DH),
        "wp8": ktile_pack(np.ascontiguousarray(Wp.T), 8).astype(BF),
        "vecs": vecs,
        "idz": np.concatenate([np.eye(128, dtype=f32),
                               np.zeros((128, 128), f32)],
                              axis=1).astype(F8),
        "dwp": dwp.astype(F8),
        "abt": abt,
        "z8d": np.zeros((128, 6272), F8),
    }
    return common


def kernel(**inputs):
    global LAST_RESULTS
    if "nc" not in _CACHE:
        _CACHE["nc"] = _build_program()
    nc = _CACHE["nc"]

    common = _prep_common(inputs)
    x = np.asarray(inputs["x"], np.float32)          # [8, 384, 28, 28]
    in_maps = []
    for c in range(B):
        m = dict(common)
        xc = x[c].reshape(3, 128, N).transpose(1, 0, 2).reshape(128, 3 * N)
        xc = np.ascontiguousarray(xc)
        m["x_c"] = xc.astype(BF)
        x8 = np.zeros((128, 4 * N), np.float32)
        x8[:, 0:3 * N] = xc
        x8[0, 3 * N:4 * N] = 1.0
        m["x8d"] = x8.astype(F8)
        in_maps.append(m)

    import os
    trace = bool(int(os.environ.get("KERNEL_TRACE", "0")))
    res = run_bass_kernel_spmd(nc, in_maps, core_ids=list(range(B)),
                               trace=trace)
    LAST_RESULTS = res
    out = np.stack([res.results[c]["out"].reshape(DIM, RES, RES)
                    for c in range(B)])
    return out.astype(np.float32)
